# revision 1
# baseline (speedup 1.0000x reference)
"""CARAFE-downsample (K=5, stride=2) Trainium2 kernel, 8-core SPMD.

Entry point: kernel(**inputs) -> np.ndarray (full (4,256,64,64) output).
Sharding: core = batch*2 + H-half; each core gets a zero-padded
(256,68,132) x-slice with 2-row/2-col halo and produces (256,32,64).
All conv weights replicated. Single compiled Bass program, run via
run_bass_kernel_spmd on cores 0-7.
"""

import numpy as np

import concourse.bacc as bacc
import concourse.mybir as mybir
import concourse.tile as tile

F32 = mybir.dt.float32
AX = mybir.AxisListType
OP = mybir.AluOpType
ACTF = mybir.ActivationFunctionType

C, CC, H, W = 256, 64, 128, 128
B = 4
HS, WS = 68, 132          # padded slice dims
HO, WO = 32, 64           # per-core output dims
NPOS = HO * WO            # 2048
K5 = 5
NT = 16                   # mask position tiles of 128


def build_nc():
    nc = bacc.Bacc("TRN2", target_bir_lowering=False, debug=False)

    xp = nc.dram_tensor("xp", [C, HS, WS], F32, kind="ExternalInput")
    w0 = nc.dram_tensor("w0", [128, CC], F32, kind="ExternalInput")
    w1 = nc.dram_tensor("w1", [128, CC], F32, kind="ExternalInput")
    wt = nc.dram_tensor("wt", [CC, 9 * 41], F32, kind="ExternalInput")
    ident = nc.dram_tensor("ident", [128, 128], F32, kind="ExternalInput")
    sel_d = nc.dram_tensor("sel", [25, 25 * 128], F32, kind="ExternalInput")
    y = nc.dram_tensor("y", [C, HO, WO], F32, kind="ExternalOutput")

    with tile.TileContext(nc) as tc:
        with (
            tc.tile_pool(name="big", bufs=1) as bigpool,
            tc.tile_pool(name="work", bufs=3) as workpool,
            tc.tile_pool(name="ps", bufs=3, space="PSUM") as ps,
        ):
            # ---- load inputs ----
            x0 = bigpool.tile([128, HS, WS], F32, tag="x0")
            x1 = bigpool.tile([128, HS, WS], F32, tag="x1")
            nc.sync.dma_start(out=x0[:], in_=xp[0:128])
            nc.sync.dma_start(out=x1[:], in_=xp[128:256])
            w0s = bigpool.tile([128, CC], F32, tag="w0")
            w1s = bigpool.tile([128, CC], F32, tag="w1")
            wts = bigpool.tile([CC, 9 * 41], F32, tag="wt")
            ids = bigpool.tile([128, 128], F32, tag="ident")
            sels = bigpool.tile([25, 25, 128], F32, tag="sel")
            nc.sync.dma_start(out=w0s[:], in_=w0[:])
            nc.sync.dma_start(out=w1s[:], in_=w1[:])
            nc.sync.dma_start(out=wts[:], in_=wt[:])
            nc.sync.dma_start(out=ids[:], in_=ident[:])
            nc.sync.dma_start(out=sels.rearrange("p a b -> p (a b)")[:],
                              in_=sel_d[:])

            x0f = x0.rearrange("p h w -> p (h w)")
            x1f = x1.rearrange("p h w -> p (h w)")

            # ---- conv1x1: cx rows 1..65 (flat offsets 132 .. 8712) ----
            cx = bigpool.tile([CC, HS, WS], F32, tag="cx")
            cxf = cx.rearrange("p h w -> p (h w)")
            base, total = WS, 65 * WS  # 8580
            CH1 = 1024
            nchunks = (total + CH1 - 1) // CH1
            for ci in range(nchunks):
                o = base + ci * CH1
                n = min(CH1, base + total - o)
                pt = ps.tile([CC, CH1], F32, tag="ps")
                for s0 in range(0, n, 512):
                    s1 = min(s0 + 512, n)
                    nc.tensor.matmul(pt[:, s0:s1], w0s[:],
                                     x0f[:, o + s0:o + s1],
                                     start=True, stop=False)
                    nc.tensor.matmul(pt[:, s0:s1], w1s[:],
                                     x1f[:, o + s0:o + s1],
                                     start=False, stop=True)
                if ci % 2 == 0:
                    nc.scalar.activation(cxf[:, o:o + n], pt[:, 0:n], ACTF.Copy)
                else:
                    nc.vector.tensor_copy(cxf[:, o:o + n], pt[:, 0:n])

            # ---- conv3x3 stride2 -> logits [41, 2048] ----
            logits = bigpool.tile([41, NPOS], F32, tag="logits")
            for c4 in range(4):  # 8 output rows per chunk
                ho0 = 8 * c4
                lgp = ps.tile([41, 512], F32, tag="ps")
                for t in range(9):
                    dy, dx = t // 3, t % 3
                    rhs = cx[:, 1 + dy + 2 * ho0: 1 + dy + 2 * ho0 + 16: 2,
                             1 + dx: 1 + dx + 128: 2]
                    nc.tensor.matmul(lgp[:], wts[:, 41 * t: 41 * (t + 1)], rhs,
                                     start=(t == 0), stop=(t == 8))
                nc.vector.tensor_copy(logits[:, 512 * c4: 512 * (c4 + 1)],
                                      lgp[:])

            # ---- transpose logits -> pos-major lgT [128, 16, 41] ----
            lgT = bigpool.tile([128, NT, 41], F32, tag="lgT")
            for t in range(NT):
                tpp = ps.tile([128, 41], F32, tag="ps")
                nc.tensor.transpose(tpp[:], logits[:, 128 * t: 128 * (t + 1)],
                                    ids[0:41, 0:41])
                nc.scalar.activation(lgT[:, t, :], tpp[:], ACTF.Copy)

            # ---- mask pipeline (pos-major) ----
            p8 = workpool.tile([128, NT, 8], F32, tag="p8")
            nc.vector.tensor_tensor(p8[:], lgT[:, :, 25:33], lgT[:, :, 33:41],
                                    OP.mult)
            p4 = workpool.tile([128, NT, 4], F32, tag="p4")
            nc.vector.tensor_tensor(p4[:], p8[:, :, 0:4], p8[:, :, 4:8], OP.mult)
            p2 = workpool.tile([128, NT, 2], F32, tag="p2")
            nc.vector.tensor_tensor(p2[:], p4[:, :, 0:2], p4[:, :, 2:4], OP.mult)
            i0 = workpool.tile([128, NT], F32, tag="i0")
            nc.vector.tensor_tensor(i0[:], p2[:, :, 0], p2[:, :, 1], OP.mult)
            ic = workpool.tile([128, NT], F32, tag="ic")
            nc.vector.tensor_scalar(ic[:], i0[:], 10.0, -10.0, OP.min, OP.max)

            mskl = workpool.tile([128, NT, 25], F32, tag="mskl")
            nc.vector.tensor_tensor(mskl[:], lgT[:, :, 0:25],
                                    ic[:].to_broadcast([128, NT, 25]), OP.mult)
            tmax = workpool.tile([128, NT], F32, tag="tmax")
            nc.vector.tensor_reduce(tmax[:], mskl[:], AX.X, OP.max)
            msub = workpool.tile([128, NT, 25], F32, tag="msub")
            nc.vector.tensor_tensor(msub[:], mskl[:],
                                    tmax[:].to_broadcast([128, NT, 25]),
                                    OP.subtract)
            mexp = workpool.tile([128, NT, 25], F32, tag="mexp")
            nc.scalar.activation(mexp[:], msub[:], ACTF.Exp)
            msum = workpool.tile([128, NT], F32, tag="msum")
            nc.vector.tensor_reduce(msum[:], mexp[:], AX.X, OP.add)
            mrec = workpool.tile([128, NT], F32, tag="mrec")
            nc.vector.reciprocal(mrec[:], msum[:])
            mskn = workpool.tile([128, NT, 25], F32, tag="mskn")
            nc.vector.tensor_tensor(mskn[:], mexp[:],
                                    mrec[:].to_broadcast([128, NT, 25]),
                                    OP.mult)

            # ---- transpose mask back to channel-major [25, 2048] ----
            mcm = bigpool.tile([25, NPOS], F32, tag="mcm")
            for t in range(NT):
                mcp = ps.tile([25, 128], F32, tag="ps")
                nc.tensor.transpose(mcp[:], mskn[:, t, :], ids[:])
                nc.scalar.activation(mcm[:, 128 * t: 128 * (t + 1)], mcp[:],
                                     ACTF.Copy)

            # ---- reassembly ----
            acc0 = bigpool.tile([128, NPOS], F32, tag="acc0")
            acc1 = bigpool.tile([128, NPOS], F32, tag="acc1")
            accs = [acc0, acc1]
            xs = [x0, x1]
            PC = 1024  # positions per chunk = 16 ho rows
            for pc in range(2):
                ho0 = 16 * pc
                for k in range(K5 * K5):
                    ky, kx = k // K5, k % K5
                    mb = ps.tile([128, PC], F32, tag="ps")
                    for nh in range(2):
                        nc.tensor.matmul(
                            mb[:, 512 * nh: 512 * (nh + 1)],
                            sels[:, k, :],
                            mcm[:, PC * pc + 512 * nh:
                                PC * pc + 512 * (nh + 1)],
                            start=True, stop=True)
                    mbv = mb.rearrange("p (a b) -> p a b", a=16)
                    for ch in range(2):
                        xsrc = xs[ch][:, 2 * ho0 + ky: 2 * ho0 + ky + 32: 2,
                                      kx: kx + 128: 2]
                        adst = accs[ch].rearrange(
                            "p (g a b) -> p g a b", g=2, a=16)[:, pc]
                        if k == 0:
                            nc.vector.tensor_tensor(adst, xsrc, mbv[:], OP.mult)
                        else:
                            tmp = workpool.tile([128, 16, 64], F32, tag="tmp")
                            nc.vector.tensor_tensor(tmp[:], xsrc, mbv[:],
                                                    OP.mult)
                            nc.vector.tensor_tensor(adst, adst, tmp[:], OP.add)

            # ---- store ----
            yf = y.rearrange("c h w -> c (h w)")
            nc.sync.dma_start(out=yf[0:128], in_=acc0[:])
            nc.sync.dma_start(out=yf[128:256], in_=acc1[:])

    nc.finalize()
    return nc


def make_core_inputs(x, w_comp, b_comp, w_enc, b_enc, w_kenc, b_kenc):
    """Full inputs -> list of 8 per-core input dicts."""
    x = np.asarray(x)
    w_compT = np.ascontiguousarray(
        np.asarray(w_comp).reshape(CC, C).T).astype(np.float32)  # [256, 64]
    we = np.asarray(w_enc)    # [25, 64, 3, 3]
    wk = np.asarray(w_kenc)   # [16, 64, 3, 3]
    wtp = np.zeros((CC, 9, 41), np.float32)
    for t in range(9):
        dy, dx = t // 3, t % 3
        wtp[:, t, 0:25] = we[:, :, dy, dx].T
        wtp[:, t, 25:41] = wk[:, :, dy, dx].T
    wtp = wtp.reshape(CC, 9 * 41)
    ident = np.eye(128, dtype=np.float32)
    sel = np.zeros((25, 25, 128), np.float32)
    for k in range(25):
        sel[k, k, :] = 1.0
    sel = np.ascontiguousarray(sel.transpose(1, 0, 2)).reshape(25, 25 * 128)

    maps = []
    for core in range(8):
        b, h = core // 2, core % 2
        start = 64 * h
        xpc = np.zeros((C, HS, WS), np.float32)
        lo, hi = start - 2, start + 66
        clo, chi = max(lo, 0), min(hi, H)
        xpc[:, clo - lo: clo - lo + (chi - clo), 2:130] = x[b, :, clo:chi, :]
        maps.append({
            "xp": xpc,
            "w0": np.ascontiguousarray(w_compT[0:128]),
            "w1": np.ascontiguousarray(w_compT[128:256]),
            "wt": wtp,
            "ident": ident,
            "sel": sel,
        })
    return maps


def assemble_output(results):
    out = np.zeros((B, C, 64, 64), np.float32)
    for core in range(8):
        b, h = core // 2, core % 2
        out[b, :, 32 * h: 32 * (h + 1), :] = results[core]["y"]
    return out


_NC_CACHE = []


def kernel(**inputs):
    import numpy as _np
    from concourse.bass_utils import run_bass_kernel_spmd

    maps = make_core_inputs(
        inputs["x"], inputs["w_comp"], inputs["b_comp"], inputs["w_enc"],
        inputs["b_enc"], inputs["w_kenc"], inputs["b_kenc"])
    if not _NC_CACHE:
        _NC_CACHE.append(build_nc())
    res = run_bass_kernel_spmd(_NC_CACHE[0], maps, list(range(8)))
    out = assemble_output(res.results)
    return out.astype(_np.float32)


# revision 2
# speedup vs baseline: 1.6597x; 1.6597x over previous
"""CARAFE-downsample (K=5, stride=2) Trainium2 kernel, 8-core SPMD.

Entry point: kernel(**inputs) -> np.ndarray (full (4,256,64,64) output).
Sharding: core = batch*2 + H-half; each core gets a zero-padded
(256,68,132) x-slice with 2-row/2-col halo and produces (256,32,64).
All conv weights replicated. Single compiled Bass program, run via
run_bass_kernel_spmd on cores 0-7.
"""

import numpy as np

import concourse.bacc as bacc
import concourse.mybir as mybir
import concourse.tile as tile

F32 = mybir.dt.float32
AX = mybir.AxisListType
OP = mybir.AluOpType
ACTF = mybir.ActivationFunctionType

C, CC, H, W = 256, 64, 128, 128
B = 4
HS, WS = 68, 132          # padded slice dims
HO, WO = 32, 64           # per-core output dims
NPOS = HO * WO            # 2048
K5 = 5
NT = 16                   # mask position tiles of 128
NPC = 2                   # position chunks
PC = NPOS // NPC          # positions per chunk
TPC = NT // NPC           # mask tiles per chunk


def build_nc():
    nc = bacc.Bacc("TRN2", target_bir_lowering=False, debug=False)

    xp = nc.dram_tensor("xp", [C, HS, WS], F32, kind="ExternalInput")
    w0 = nc.dram_tensor("w0", [128, CC], F32, kind="ExternalInput")
    w1 = nc.dram_tensor("w1", [128, CC], F32, kind="ExternalInput")
    wt = nc.dram_tensor("wt", [CC, 9 * 41], F32, kind="ExternalInput")
    ident = nc.dram_tensor("ident", [128, 128], F32, kind="ExternalInput")
    mscr = nc.dram_tensor("mscr", [25, NPOS], F32)
    y = nc.dram_tensor("y", [C, HO, WO], F32, kind="ExternalOutput")

    with tile.TileContext(nc) as tc:
        with (
            tc.tile_pool(name="big", bufs=1) as bigpool,
            tc.tile_pool(name="work", bufs=3) as workpool,
            tc.tile_pool(name="mbp", bufs=4) as mbpool,
            tc.tile_pool(name="ps", bufs=3, space="PSUM") as ps,
        ):
            # ---- load inputs ----
            x0 = bigpool.tile([128, HS, WS], F32, tag="x0")
            x1 = bigpool.tile([128, HS, WS], F32, tag="x1")
            nc.sync.dma_start(out=x0[:], in_=xp[0:128])
            nc.sync.dma_start(out=x1[:], in_=xp[128:256])
            w0s = bigpool.tile([128, CC], F32, tag="w0")
            w1s = bigpool.tile([128, CC], F32, tag="w1")
            wts = bigpool.tile([CC, 9 * 41], F32, tag="wt")
            ids = bigpool.tile([128, 128], F32, tag="ident")
            nc.sync.dma_start(out=w0s[:], in_=w0[:])
            nc.sync.dma_start(out=w1s[:], in_=w1[:])
            nc.sync.dma_start(out=wts[:], in_=wt[:])
            nc.sync.dma_start(out=ids[:], in_=ident[:])

            x0f = x0.rearrange("p h w -> p (h w)")
            x1f = x1.rearrange("p h w -> p (h w)")

            cx = bigpool.tile([CC, HS, WS], F32, tag="cx")
            cxf = cx.rearrange("p h w -> p (h w)")
            logits = bigpool.tile([41, NPOS], F32, tag="logits")
            mcm = bigpool.tile([25, NPOS], F32, tag="mcm")
            acc0 = bigpool.tile([128, NPOS], F32, tag="acc0")
            acc1 = bigpool.tile([128, NPOS], F32, tag="acc1")
            accs = [acc0, acc1]
            xs = [x0, x1]

            copy_flip = [0]

            def psum_copy(dst, src):
                # alternate PSUM->SBUF copies between ScalarE and DVE
                if copy_flip[0] % 2 == 0:
                    nc.scalar.activation(dst, src, ACTF.Copy)
                else:
                    nc.vector.tensor_copy(dst, src)
                copy_flip[0] += 1

            for pc in range(NPC):
                ho0 = TPC * 2 * pc          # first output row of chunk
                # ---- conv1x1 for cx rows needed by this chunk ----
                # chunk pc needs cx rows [2*ho0+1 .. 2*ho0+33] -> 33 rows
                r0 = 2 * ho0 + 1
                o0, o1 = r0 * WS, (r0 + 33) * WS  # 4356 flat elems
                CH1 = 1024
                o = o0
                while o < o1:
                    n = min(CH1, o1 - o)
                    pt = ps.tile([CC, CH1], F32, tag="ps")
                    for s0 in range(0, n, 512):
                        s1 = min(s0 + 512, n)
                        nc.tensor.matmul(pt[:, s0:s1], w0s[:],
                                         x0f[:, o + s0:o + s1],
                                         start=True, stop=False)
                        nc.tensor.matmul(pt[:, s0:s1], w1s[:],
                                         x1f[:, o + s0:o + s1],
                                         start=False, stop=True)
                    psum_copy(cxf[:, o:o + n], pt[:, 0:n])
                    o += n

                # ---- conv3x3 stride2 -> logits for this chunk ----
                for c4 in range(2 * pc, 2 * pc + 2):  # 8 output rows per c4
                    hoc = 8 * c4
                    lgp = ps.tile([41, 512], F32, tag="ps")
                    for t in range(9):
                        dy, dx = t // 3, t % 3
                        rhs = cx[:, 1 + dy + 2 * hoc: 1 + dy + 2 * hoc + 16: 2,
                                 1 + dx: 1 + dx + 128: 2]
                        nc.tensor.matmul(lgp[:], wts[:, 41 * t: 41 * (t + 1)],
                                         rhs, start=(t == 0), stop=(t == 8))
                    nc.vector.tensor_copy(logits[:, 512 * c4: 512 * (c4 + 1)],
                                          lgp[:])

                # ---- transpose logits -> pos-major lgT [128, TPC, 41] ----
                lgT = workpool.tile([128, TPC, 41], F32, tag="lgT")
                for tt in range(TPC):
                    t = TPC * pc + tt
                    tpp = ps.tile([128, 41], F32, tag="ps")
                    nc.tensor.transpose(tpp[:],
                                        logits[:, 128 * t: 128 * (t + 1)],
                                        ids[0:41, 0:41])
                    nc.scalar.activation(lgT[:, tt, :], tpp[:], ACTF.Copy)

                # ---- mask pipeline (pos-major) ----
                p8 = workpool.tile([128, TPC, 8], F32, tag="p8")
                nc.vector.tensor_tensor(p8[:], lgT[:, :, 25:33],
                                        lgT[:, :, 33:41], OP.mult)
                p4 = workpool.tile([128, TPC, 4], F32, tag="p4")
                nc.vector.tensor_tensor(p4[:], p8[:, :, 0:4], p8[:, :, 4:8],
                                        OP.mult)
                p2 = workpool.tile([128, TPC, 2], F32, tag="p2")
                nc.vector.tensor_tensor(p2[:], p4[:, :, 0:2], p4[:, :, 2:4],
                                        OP.mult)
                i0 = workpool.tile([128, TPC], F32, tag="i0")
                nc.vector.tensor_tensor(i0[:], p2[:, :, 0], p2[:, :, 1],
                                        OP.mult)
                ic = workpool.tile([128, TPC], F32, tag="ic")
                nc.vector.tensor_scalar(ic[:], i0[:], 10.0, -10.0,
                                        OP.min, OP.max)

                mskl = workpool.tile([128, TPC, 25], F32, tag="mskl")
                nc.vector.tensor_tensor(mskl[:], lgT[:, :, 0:25],
                                        ic[:].to_broadcast([128, TPC, 25]),
                                        OP.mult)
                tmax = workpool.tile([128, TPC], F32, tag="tmax")
                nc.vector.tensor_reduce(tmax[:], mskl[:], AX.X, OP.max)
                msub = workpool.tile([128, TPC, 25], F32, tag="msub")
                nc.vector.tensor_tensor(msub[:], mskl[:],
                                        tmax[:].to_broadcast([128, TPC, 25]),
                                        OP.subtract)
                mexp = workpool.tile([128, TPC, 25], F32, tag="mexp")
                nc.scalar.activation(mexp[:], msub[:], ACTF.Exp)
                msum = workpool.tile([128, TPC], F32, tag="msum")
                nc.vector.tensor_reduce(msum[:], mexp[:], AX.X, OP.add)
                mrec = workpool.tile([128, TPC], F32, tag="mrec")
                nc.vector.reciprocal(mrec[:], msum[:])
                mskn = workpool.tile([128, TPC, 25], F32, tag="mskn")
                nc.vector.tensor_tensor(mskn[:], mexp[:],
                                        mrec[:].to_broadcast([128, TPC, 25]),
                                        OP.mult)

                # ---- transpose mask back to channel-major ----
                for tt in range(TPC):
                    t = TPC * pc + tt
                    mcp = ps.tile([25, 128], F32, tag="ps")
                    nc.tensor.transpose(mcp[:], mskn[:, tt, :], ids[:])
                    nc.scalar.activation(mcm[:, 128 * t: 128 * (t + 1)],
                                         mcp[:], ACTF.Copy)

                # ---- bounce mask chunk to DRAM for replicating DMAs ----
                nc.sync.dma_start(out=mscr[:, PC * pc: PC * (pc + 1)],
                                  in_=mcm[:, PC * pc: PC * (pc + 1)])

                # ---- reassembly for this chunk ----
                for k in range(K5 * K5):
                    ky, kx = k // K5, k % K5
                    mb = mbpool.tile([128, PC], F32, tag="mb")
                    nc.sync.dma_start(
                        out=mb[:],
                        in_=mscr[k: k + 1,
                                 PC * pc: PC * (pc + 1)].to_broadcast([128, PC]))
                    mbv = mb.rearrange("p (a b) -> p a b", a=PC // 64)
                    for ch in range(2):
                        eng = nc.vector if (2 * k + ch) % 3 != 2 else nc.gpsimd
                        xsrc = xs[ch][:, 2 * ho0 + ky: 2 * ho0 + ky + 32: 2,
                                      kx: kx + 128: 2]
                        adst = accs[ch].rearrange(
                            "p (g a b) -> p g a b", g=NPC, a=PC // 64)[:, pc]
                        if k == 0:
                            eng.tensor_tensor(adst, xsrc, mbv[:], OP.mult)
                        else:
                            tmp = workpool.tile([128, PC // 64, 64], F32,
                                                tag=f"tmp{ch}")
                            eng.tensor_tensor(tmp[:], xsrc, mbv[:], OP.mult)
                            eng.tensor_tensor(adst, adst, tmp[:], OP.add)

            # ---- store ----
            yf = y.rearrange("c h w -> c (h w)")
            nc.sync.dma_start(out=yf[0:128], in_=acc0[:])
            nc.sync.dma_start(out=yf[128:256], in_=acc1[:])

    nc.finalize()
    return nc


def make_core_inputs(x, w_comp, b_comp, w_enc, b_enc, w_kenc, b_kenc):
    """Full inputs -> list of 8 per-core input dicts."""
    x = np.asarray(x)
    w_compT = np.ascontiguousarray(
        np.asarray(w_comp).reshape(CC, C).T).astype(np.float32)  # [256, 64]
    we = np.asarray(w_enc)    # [25, 64, 3, 3]
    wk = np.asarray(w_kenc)   # [16, 64, 3, 3]
    wtp = np.zeros((CC, 9, 41), np.float32)
    for t in range(9):
        dy, dx = t // 3, t % 3
        wtp[:, t, 0:25] = we[:, :, dy, dx].T
        wtp[:, t, 25:41] = wk[:, :, dy, dx].T
    wtp = wtp.reshape(CC, 9 * 41)
    ident = np.eye(128, dtype=np.float32)

    maps = []
    for core in range(8):
        b, h = core // 2, core % 2
        start = 64 * h
        xpc = np.zeros((C, HS, WS), np.float32)
        lo, hi = start - 2, start + 66
        clo, chi = max(lo, 0), min(hi, H)
        xpc[:, clo - lo: clo - lo + (chi - clo), 2:130] = x[b, :, clo:chi, :]
        maps.append({
            "xp": xpc,
            "w0": np.ascontiguousarray(w_compT[0:128]),
            "w1": np.ascontiguousarray(w_compT[128:256]),
            "wt": wtp,
            "ident": ident,
        })
    return maps


def assemble_output(results):
    out = np.zeros((B, C, 64, 64), np.float32)
    for core in range(8):
        b, h = core // 2, core % 2
        out[b, :, 32 * h: 32 * (h + 1), :] = results[core]["y"]
    return out


_NC_CACHE = []


def kernel(**inputs):
    import numpy as _np
    from concourse.bass_utils import run_bass_kernel_spmd

    maps = make_core_inputs(
        inputs["x"], inputs["w_comp"], inputs["b_comp"], inputs["w_enc"],
        inputs["b_enc"], inputs["w_kenc"], inputs["b_kenc"])
    if not _NC_CACHE:
        _NC_CACHE.append(build_nc())
    res = run_bass_kernel_spmd(_NC_CACHE[0], maps, list(range(8)))
    out = assemble_output(res.results)
    return out.astype(_np.float32)
